# revision 1
# baseline (speedup 1.0000x reference)
"""Trainium2 Bass kernel for a dense Mamba (selective-scan) block, SPMD over 8 NeuronCores.

Sharding: tensor-parallel over d_inner (2048 -> 256 channels/core).
Per core: in_proj (bf16 matmul) -> depthwise causal conv via 4 diagonal matmuls on
TensorE w/ PSUM accumulation -> SiLU (fused conv bias, ScalarE) -> x_proj partial ->
chunked AllReduce (3.1MB) -> dt_proj + fused softplus(+bias) -> selective scan:
per (t-chunk, state-index n): dA = Exp(dt * A[:,n]) via ScalarE per-partition scale,
dBx = dtx * broadcast(B_n) (VectorE bf16 2x), hardware tensor_tensor_scan
(fp32 state), y_n = h * broadcast(C_n), n-reduction via identity-matmul PSUM
accumulation -> D-skip + SiLU(z) gate -> AllToAll of gated activations (3.7MB)
-> full out_proj per t-slice locally -> per-core output slice, host concat.

Shapes hardcoded for: B=2, L=4096, d_model=1024, d_inner=2048, d_state=16,
d_conv=4, dt_rank=64, f32 I/O.
"""
import numpy as np
import ml_dtypes
from contextlib import ExitStack

import concourse.bass as bass
import concourse.bacc as bacc
import concourse.tile as tile
from concourse import mybir
from concourse import bass_utils

BF = ml_dtypes.bfloat16
F32 = mybir.dt.float32
BF16 = mybir.dt.bfloat16

NCORES = 8
B, L, DM = 2, 4096, 1024
DI, DS, DC, DTR = 2048, 16, 4, 64
DL = DI // NCORES          # 256 local channels
NDH = DL // 128            # 2 d-half tiles
T = B * L                  # 8192 flattened (b, l)
TSL = T // NCORES          # 1024 t-slice per core for the output
TCA = 512                  # phase A/B t-chunk
TCC = 1024                 # phase C scan t-chunk
NTCB = T // TCC            # 8
NQAR = 4                   # AllReduce chunks

_cached = {}


def _build():
    nc = bacc.Bacc("TRN2", target_bir_lowering=False, num_devices=NCORES)

    # ---- I/O -------------------------------------------------------------
    d_hT = nc.dram_tensor("hT", (DM, T), BF16, kind="ExternalInput")
    d_wxzT = nc.dram_tensor("wxzT", (DM, 2 * DL), BF16, kind="ExternalInput")
    d_cdiag = nc.dram_tensor("cdiag", (DC, NDH, 128, 128), BF16, kind="ExternalInput")
    d_convb = nc.dram_tensor("convb", (NDH, 128, 1), F32, kind="ExternalInput")
    d_xprojT = nc.dram_tensor("xprojT", (NDH, 128, DTR + 2 * DS), BF16, kind="ExternalInput")
    d_dtwT = nc.dram_tensor("dtwT", (DTR, DL), BF16, kind="ExternalInput")
    d_dtb = nc.dram_tensor("dtb", (NDH, 128, 1), F32, kind="ExternalInput")
    d_aneg = nc.dram_tensor("aneg", (NDH, 128, DS), F32, kind="ExternalInput")
    d_dvec = nc.dram_tensor("dvec", (NDH, 128, 1), F32, kind="ExternalInput")
    d_woutT = nc.dram_tensor("woutT", (2 * NCORES, 128, DM), BF16, kind="ExternalInput")
    d_ident = nc.dram_tensor("ident", (128, 128), BF16, kind="ExternalInput")
    d_out = nc.dram_tensor("out_slice", (TSL, DM), F32, kind="ExternalOutput")

    # ---- internal DRAM ---------------------------------------------------
    d_zsp = nc.dram_tensor("zsp", (NDH, 128, T), BF16, kind="Internal")
    d_xssp = nc.dram_tensor("xssp", (NDH, 128, T), BF16, kind="Internal")
    # x_dbl partials, chunk-major for chunked AllReduce
    d_xdp = nc.dram_tensor("xdp", (NQAR, DTR + 2 * DS, T // NQAR), F32, kind="Internal")
    d_xd = nc.dram_tensor("xd", (NQAR, DTR + 2 * DS, T // NQAR), F32, kind="Internal",
                          addr_space="Shared")
    d_bc = nc.dram_tensor("bcrows", (2 * DS, T), BF16, kind="Internal")
    d_a2ai = nc.dram_tensor("a2ai", (NCORES, DL, TSL), BF16, kind="Internal")
    d_a2ao = nc.dram_tensor("a2ao", (NCORES, DL, TSL), BF16, kind="Internal")

    groups = [list(range(NCORES))]

    with tile.TileContext(nc) as tc, ExitStack() as ctx:
        consts = ctx.enter_context(tc.tile_pool(name="consts", bufs=1))
        arena = ctx.enter_context(tc.tile_pool(name="arena", bufs=3))
        work = ctx.enter_context(tc.tile_pool(name="work", bufs=2))
        work2 = ctx.enter_context(tc.tile_pool(name="work2", bufs=2))
        psA = ctx.enter_context(tc.tile_pool(name="psA", bufs=4, space="PSUM"))
        psY = ctx.enter_context(tc.tile_pool(name="psY", bufs=4, space="PSUM"))

        # ---- load constants ----------------------------------------------
        wxz = consts.tile([128, 8, 2 * DL], BF16, tag="wxz")
        nc.sync.dma_start(out=wxz, in_=d_wxzT[:, :].rearrange("(k p) m -> p k m", p=128))
        cdg = consts.tile([128, DC, NDH, 128], BF16, tag="cdg")
        nc.sync.dma_start(
            out=cdg, in_=bass.AP(tensor=d_cdiag[:, :, :, :].tensor, offset=0,
                                 ap=[[128, 128], [NDH * 128 * 128, DC], [128 * 128, NDH], [1, 128]]))
        convb = consts.tile([128, NDH, 1], F32, tag="convb")
        nc.sync.dma_start(out=convb, in_=d_convb[:, :, :].rearrange("h p one -> p h one"))
        xprj = consts.tile([128, NDH, DTR + 2 * DS], BF16, tag="xprj")
        nc.sync.dma_start(out=xprj, in_=d_xprojT[:, :, :].rearrange("h p m -> p h m"))
        dtw = consts.tile([DTR, DL], BF16, tag="dtw")
        nc.sync.dma_start(out=dtw, in_=d_dtwT[:, :])
        dtb = consts.tile([128, NDH, 1], F32, tag="dtb")
        nc.sync.dma_start(out=dtb, in_=d_dtb[:, :, :].rearrange("h p one -> p h one"))
        aneg = consts.tile([128, NDH, DS], F32, tag="aneg")
        nc.sync.dma_start(out=aneg, in_=d_aneg[:, :, :].rearrange("h p n -> p h n"))
        dvec = consts.tile([128, NDH, 1], F32, tag="dvec")
        nc.sync.dma_start(out=dvec, in_=d_dvec[:, :, :].rearrange("h p one -> p h one"))
        ident = consts.tile([128, 128], BF16, tag="ident")
        nc.sync.dma_start(out=ident, in_=d_ident[:, :])
        carry = consts.tile([128, NDH, DS], F32, tag="carry")

        # big sequential-lifetime activations share one arena tag:
        #   xpad (dies after conv) -> xs, dts, dtx (live into phase C)
        xpad = arena.tile([128, NDH, B, 3 + L], BF16, tag="arena")
        xs = arena.tile([128, NDH, T], BF16, tag="arena")

        for h in range(NDH):
            for b in range(B):
                nc.vector.memset(xpad[:, h, b, 0:3], 0.0)

        # ---- Phase A: in_proj --------------------------------------------
        NTA = T // TCA  # 16
        for t in range(NTA):
            ht = work.tile([128, 8, TCA], BF16, tag="ht")
            nc.sync.dma_start(
                out=ht,
                in_=bass.AP(tensor=d_hT[:, :].tensor, offset=t * TCA,
                            ap=[[T, 128], [128 * T, 8], [1, TCA]]))
            b, l0 = (t * TCA) // L, (t * TCA) % L
            for m in range(4):  # 0,1: x halves; 2,3: z halves
                pxz = psA.tile([128, TCA], F32, tag="ps")
                for k in range(8):
                    nc.tensor.matmul(pxz, lhsT=wxz[:, k, m * 128:(m + 1) * 128],
                                     rhs=ht[:, k, :], start=(k == 0), stop=(k == 7))
                if m < 2:
                    nc.scalar.copy(xpad[:, m, b, 3 + l0: 3 + l0 + TCA], pxz)
                else:
                    zt = work.tile([128, TCA], BF16, tag="zt")
                    nc.scalar.copy(zt, pxz)
                    zs = work.tile([128, TCA], BF16, tag="zs")
                    nc.scalar.activation(zs, pxz, mybir.ActivationFunctionType.Sigmoid)
                    nc.vector.tensor_mul(zt, zt, zs)
                    nc.sync.dma_start(out=d_zsp[m - 2, :, t * TCA:(t + 1) * TCA], in_=zt)

        # ---- Phase A2: conv (4 diag matmuls) + SiLU; x_proj partial ------
        for h in range(NDH):
            for b in range(B):
                for c in range(L // TCA):  # 8 chunks
                    l0 = c * TCA
                    pc = psA.tile([128, TCA], F32, tag="ps")
                    for j in range(DC):
                        nc.tensor.matmul(pc, lhsT=cdg[:, j, h, :],
                                         rhs=xpad[:, h, b, l0 + j: l0 + j + TCA],
                                         start=(j == 0), stop=(j == DC - 1))
                    t0 = b * L + l0
                    xpre = work.tile([128, TCA], BF16, tag="xpre")
                    nc.scalar.activation(xpre, pc,
                                         mybir.ActivationFunctionType.Identity,
                                         bias=convb[:, h, 0:1], scale=1.0)
                    xsg = work.tile([128, TCA], BF16, tag="xsg")
                    nc.scalar.activation(xsg, pc,
                                         mybir.ActivationFunctionType.Sigmoid,
                                         bias=convb[:, h, 0:1], scale=1.0)
                    nc.vector.tensor_mul(xs[:, h, t0:t0 + TCA], xpre, xsg)
                    nc.sync.dma_start(out=d_xssp[h, :, t0:t0 + TCA],
                                      in_=xs[:, h, t0:t0 + TCA])
        # x_proj partials (contraction over local d)
        for t in range(NTA):
            t0 = t * TCA
            pxp = psA.tile([96, TCA], F32, tag="ps")
            for h in range(NDH):
                nc.tensor.matmul(pxp, lhsT=xprj[:, h, :], rhs=xs[:, h, t0:t0 + TCA],
                                 start=(h == 0), stop=(h == NDH - 1))
            xpt = work.tile([96, TCA], F32, tag="xpt")
            nc.scalar.copy(xpt, pxp)
            q, qo = t0 // (T // NQAR), t0 % (T // NQAR)
            nc.sync.dma_start(out=d_xdp[q, :, qo:qo + TCA], in_=xpt)

        # ---- Phase A3: chunked AllReduce of x_dbl partials ---------------
        for q in range(NQAR):
            nc.gpsimd.collective_compute(
                kind="AllReduce", op=mybir.AluOpType.add, replica_groups=groups,
                ins=[d_xdp[q, :, :]], outs=[d_xd[q, :, :]])

        # ---- Phase B: dt_proj + softplus; dtx; B/C rows to bf16 ----------
        dts = arena.tile([128, NDH, T], BF16, tag="arena")
        dtx = arena.tile([128, NDH, T], BF16, tag="arena")
        for t in range(NTA):
            t0 = t * TCA
            q, qo = t0 // (T // NQAR), t0 % (T // NQAR)
            xdt = work.tile([96, TCA], F32, tag="xdt")
            nc.sync.dma_start(out=xdt, in_=d_xd[q, :, qo:qo + TCA])
            xdb = work.tile([96, TCA], BF16, tag="xdb")
            nc.vector.tensor_copy(xdb, xdt)
            # B, C rows -> bf16 compact DRAM for later broadcast
            nc.sync.dma_start(out=d_bc[:, t0:t0 + TCA], in_=xdb[DTR:DTR + 2 * DS, :])
            for h in range(NDH):
                pdt = psA.tile([128, TCA], F32, tag="ps")
                nc.tensor.matmul(pdt, lhsT=dtw[:, h * 128:(h + 1) * 128],
                                 rhs=xdb[0:DTR, :], start=True, stop=True)
                spe = work.tile([128, TCA], F32, tag="spe")
                nc.scalar.activation(spe, pdt, mybir.ActivationFunctionType.Exp,
                                     bias=dtb[:, h, 0:1], scale=1.0)
                nc.scalar.activation(dts[:, h, t0:t0 + TCA], spe,
                                     mybir.ActivationFunctionType.Ln,
                                     bias=1.0, scale=1.0)
                nc.vector.tensor_mul(dtx[:, h, t0:t0 + TCA],
                                     dts[:, h, t0:t0 + TCA], xs[:, h, t0:t0 + TCA])

        # ---- Phase C: selective scan over (tcb, n, dh) -------------------
        for tcb in range(NTCB):
            t0 = tcb * TCC
            pys = [[psY.tile([128, 512], F32, tag="py", name=f"pys_{tcb}_{h2}_{q2}")
                    for q2 in range(TCC // 512)] for h2 in range(NDH)]
            for n in range(DS):
                bbc = work2.tile([128, TCC], BF16, tag="bbc")
                nc.sync.dma_start(
                    out=bbc, in_=bass.AP(tensor=d_bc[:, :].tensor, offset=n * T + t0,
                                         ap=[[0, 128], [1, TCC]]))
                cbc = work2.tile([128, TCC], BF16, tag="cbc")
                nc.sync.dma_start(
                    out=cbc, in_=bass.AP(tensor=d_bc[:, :].tensor,
                                         offset=(DS + n) * T + t0,
                                         ap=[[0, 128], [1, TCC]]))
                for h in range(NDH):
                    dA = work2.tile([128, TCC], F32, tag="dA")
                    nc.scalar.activation(dA, dts[:, h, t0:t0 + TCC],
                                         mybir.ActivationFunctionType.Exp,
                                         bias=0.0, scale=aneg[:, h, n:n + 1])
                    dBx = work2.tile([128, TCC], BF16, tag="dBx")
                    nc.vector.tensor_mul(dBx, dtx[:, h, t0:t0 + TCC], bbc)
                    hts = work2.tile([128, TCC], BF16, tag="hts")
                    init = 0.0 if (t0 % L == 0) else carry[:, h, n:n + 1]
                    nc.vector.tensor_tensor_scan(
                        out=hts, data0=dA, data1=dBx, initial=init,
                        op0=mybir.AluOpType.mult, op1=mybir.AluOpType.add)
                    if (t0 + TCC) % L != 0:
                        nc.vector.tensor_copy(carry[:, h, n:n + 1], hts[:, TCC - 1:TCC])
                    yp = work2.tile([128, TCC], BF16, tag="yp")
                    nc.vector.tensor_mul(yp, hts, cbc)
                    for qq in range(TCC // 512):
                        nc.tensor.matmul(pys[h][qq], lhsT=ident,
                                         rhs=yp[:, qq * 512:(qq + 1) * 512],
                                         start=(n == 0), stop=(n == DS - 1))
            # gate + write A2A input
            for h in range(NDH):
                ys = work2.tile([128, TCC], BF16, tag="ys")
                for qq in range(TCC // 512):
                    nc.scalar.copy(ys[:, qq * 512:(qq + 1) * 512], pys[h][qq])
                sz = work2.tile([128, TCC], BF16, tag="sz")
                nc.sync.dma_start(out=sz, in_=d_zsp[h, :, t0:t0 + TCC])
                xst = work2.tile([128, TCC], BF16, tag="xst")
                nc.sync.dma_start(out=xst, in_=d_xssp[h, :, t0:t0 + TCC])
                # in-place: xst = xst * D ; ys = ys + xst ; ys = ys * sz
                nc.vector.tensor_scalar(out=xst, in0=xst, scalar1=dvec[:, h, 0:1],
                                        scalar2=None, op0=mybir.AluOpType.mult)
                nc.vector.tensor_add(ys, ys, xst)
                nc.vector.tensor_mul(ys, ys, sz)
                for j2 in range(TCC // TSL):
                    jsh = (t0 // TSL) + j2
                    nc.sync.dma_start(out=d_a2ai[jsh, h * 128:(h + 1) * 128, :],
                                      in_=ys[:, j2 * TSL:(j2 + 1) * TSL])

        # ---- Phase D: AllToAll -------------------------------------------
        nc.gpsimd.collective_compute(
            kind="AllToAll", op=mybir.AluOpType.bypass, replica_groups=groups,
            ins=[d_a2ai[:, :, :]], outs=[d_a2ao[:, :, :]])

        # ---- Phase E: full out_proj on local t-slice (streamed weights) --
        for tsg in range(4):  # groups of 2 t-subtiles of 128
            pos = [[psY.tile([128, 512], F32, tag="py", name=f"pos_{tsg}_{t3}_{f3}")
                    for f3 in range(2)] for t3 in range(2)]
            for kt in range(16):
                i, h = kt // 2, kt % 2
                ykt = work.tile([128, TSL], BF16, tag="ykS")
                nc.sync.dma_start(out=ykt, in_=d_a2ao[i, h * 128:(h + 1) * 128, :])
                wot = work.tile([128, DM], BF16, tag="woS")
                nc.sync.dma_start(out=wot, in_=d_woutT[kt, :, :])
                for t2 in range(2):
                    ts = tsg * 2 + t2
                    for fh in range(2):
                        nc.tensor.matmul(pos[t2][fh],
                                         lhsT=ykt[:, ts * 128:(ts + 1) * 128],
                                         rhs=wot[:, fh * 512:(fh + 1) * 512],
                                         start=(kt == 0), stop=(kt == 15))
            for t2 in range(2):
                ts = tsg * 2 + t2
                for fh in range(2):
                    ot = work.tile([128, 512], F32, tag="otS")
                    nc.scalar.copy(ot, pos[t2][fh])
                    nc.sync.dma_start(
                        out=d_out[ts * 128:(ts + 1) * 128, fh * 512:(fh + 1) * 512],
                        in_=ot)

    nc.compile()
    return nc


def _host_prep(inputs):
    """Per-core input maps from full inputs (layout prep + bf16 casts only)."""
    hs = np.asarray(inputs["hidden_states"], np.float32)
    wxz = np.asarray(inputs["in_proj_w"], np.float32)
    cw = np.asarray(inputs["conv_w"], np.float32)
    cb = np.asarray(inputs["conv_b"], np.float32)
    xpw = np.asarray(inputs["x_proj_w"], np.float32)
    dpw = np.asarray(inputs["dt_proj_w"], np.float32)
    dpb = np.asarray(inputs["dt_proj_b"], np.float32)
    alog = np.asarray(inputs["A_log"], np.float32)
    dv = np.asarray(inputs["D"], np.float32)
    wo = np.asarray(inputs["out_proj_w"], np.float32)

    hT = np.ascontiguousarray(hs.reshape(T, DM).T).astype(BF)
    woutT = np.ascontiguousarray(wo.T).reshape(2 * NCORES, 128, DM).astype(BF)
    ident = np.eye(128, dtype=np.float32).astype(BF)

    in_maps = []
    for i in range(NCORES):
        lo = i * DL
        sl = slice(lo, lo + DL)
        wxzT = np.ascontiguousarray(
            np.concatenate([wxz[sl], wxz[DI + lo:DI + lo + DL]], axis=0).T).astype(BF)
        cdiag = np.zeros((DC, NDH, 128, 128), np.float32)
        for j in range(DC):
            for h in range(NDH):
                np.fill_diagonal(cdiag[j, h], cw[lo + h * 128:lo + (h + 1) * 128, j])
        in_maps.append({
            "hT": hT,
            "wxzT": wxzT,
            "cdiag": cdiag.astype(BF),
            "convb": cb[sl].reshape(NDH, 128, 1),
            "xprojT": np.ascontiguousarray(xpw[:, sl].T).reshape(NDH, 128, 96).astype(BF),
            "dtwT": np.ascontiguousarray(dpw[sl].T).astype(BF),
            "dtb": dpb[sl].reshape(NDH, 128, 1),
            "aneg": (-np.exp(alog[sl])).reshape(NDH, 128, DS).astype(np.float32),
            "dvec": dv[sl].reshape(NDH, 128, 1),
            "woutT": woutT,
            "ident": ident,
        })
    return in_maps


def _run(inputs, trace=False, **kw):
    if "nc" not in _cached:
        _cached["nc"] = _build()
    nc = _cached["nc"]
    in_maps = _host_prep(inputs)
    res = bass_utils.run_bass_kernel_spmd(
        nc, in_maps, core_ids=list(range(NCORES)), trace=trace, **kw)
    out = np.concatenate([res.results[i]["out_slice"] for i in range(NCORES)], axis=0)
    return out.reshape(B, L, DM).astype(np.float32), res


def kernel(**inputs):
    out, _ = _run(inputs, trace=False)
    return out

